# revision 7
# baseline (speedup 1.0000x reference)
"""Trainium2 Bass kernel for 2-layer GraphSAGE (mean aggregator), 8 NeuronCores.

Sharding: layer 0 dst-sharded (feat replicated, per-core edge buckets, local
dma_gather + one-hot matmul segment-sum); layer 1 src-sharded with a single
ReduceScatter of partial message sums; final matmuls on the dst owner.

Layout: gather slots are range-major with <=1024-row gather windows (SWDGE
descriptor-ring limit); 128-slot chunks may span two dst blocks, handled by
sequence-parity-encoded dst positions and a second masked matmul.
"""
import os
import sys

sys.path.insert(0, "/opt/trn_rl_repo")

import numpy as np

N_SRC0, N_DST0 = 200000, 40000
N_SRC1, N_DST1 = 40000, 8000
E0, E1 = 1000000, 80000
D_IN, D_HID, D_OUT = 256, 512, 256
C = 8                      # cores
RANGE = 32768              # int16 gather index range
NR0 = (N_SRC0 + RANGE - 1) // RANGE   # 7 src ranges, layer 0
PAD_DST1 = 8064            # 63 * 128
SH1 = PAD_DST1 // C        # 1008 rows per core after ReduceScatter
NB0 = 40                   # local dst blocks of 128 (5120 padded local rows)
NLOC = 5000                # real local dst rows, layer 0
NB1 = PAD_DST1 // 128      # 63 global dst blocks, layer 1
NBF = 8                    # final blocks over 1008 rows (7*128 + 112)
WIN = 1024                 # max gather rows per instruction (ring limit)

LAST_RESULT = None  # BassKernelResults of the most recent run (for test.py)


def _wrap_idx(slots):
    """int16 slot list (len % 16 == 0) -> [128, len//16] wrapped/replicated."""
    n = len(slots)
    w = slots.reshape(n // 16, 16).T            # [16, n//16]
    return np.tile(w, (8, 1)).astype(np.int16)  # [128, n//16]


def _layout_ranges(counts, n_ranges, nblocks):
    """Build range-major slot layout.

    counts: [nblocks, n_ranges] slot count per (block, range) segment.
    Returns: seg_off[b,k] (global slot offset), rng_off[k], rng_len[k]
    (padded to 128), TOT, windows: list of (k, slot_start, n_slots),
    submm: per global chunk, list of (b, par, start, stop).
    """
    seg_off = np.zeros((nblocks, n_ranges), np.int64)
    rng_off = np.zeros(n_ranges, np.int64)
    rng_len = np.zeros(n_ranges, np.int64)
    pos = 0
    submm = {}
    windows = []
    for k in range(n_ranges):
        rng_off[k] = pos
        seq = 0
        for b in range(nblocks):
            n = int(counts[b, k])
            if n == 0:
                continue
            seg_off[b, k] = pos
            par = seq % 8
            c0, c1 = pos // 128, (pos + n - 1) // 128
            for q in range(c0, c1 + 1):
                submm.setdefault(q, []).append(
                    [b, par, q == c0, q == c1])
            pos += n
            seq += 1
        pos = (pos + 127) // 128 * 128
        rng_len[k] = pos - rng_off[k]
        s = 0
        while s < rng_len[k]:
            w = min(WIN, rng_len[k] - s)
            windows.append((k, int(rng_off[k] + s), int(w)))
            s += w
    return seg_off, rng_off, rng_len, int(pos), windows, submm


def _host_prep(feat, Wself0, Wneigh0, b0, Wself1, Wneigh1, b1,
               edge_src0, edge_dst0, edge_src1, edge_dst1):
    import heapq

    src0 = np.asarray(edge_src0).astype(np.int64)
    dst0 = np.asarray(edge_dst0).astype(np.int64)
    src1 = np.asarray(edge_src1).astype(np.int64)
    dst1 = np.asarray(edge_dst1).astype(np.int64)
    feat = np.asarray(feat, dtype=np.float32)

    # ---- ownership of layer-0 dst rows (also layer-1 src rows) ----
    s_core = np.full(C, SH1, np.int64)
    s_core[C - 1] = N_DST1 - SH1 * (C - 1)          # 944
    remB = NLOC - s_core
    baseB = N_DST1 + np.concatenate(([0], np.cumsum(remB)[:-1]))
    deg0_i = np.bincount(dst0, minlength=N_DST0)
    own = np.empty(N_DST0, np.int64)
    loc = np.empty(N_DST0, np.int64)
    local2global = []
    for c in range(C):
        ga = np.arange(SH1 * c, SH1 * c + s_core[c])
        gb = np.arange(baseB[c], baseB[c] + remB[c])
        # LPT-balance the non-pinned rows (locals >= s_c) across blocks so
        # per-block edge counts equalize -> less cross-core bucket padding.
        nA = int(s_core[c])
        cap = np.zeros(NB0, np.int64)
        tot = np.zeros(NB0, np.int64)
        for b in range(NB0):
            lo, hi = b * 128, min((b + 1) * 128, NLOC)
            if hi <= lo:
                continue
            pinned = max(0, min(hi, nA) - lo)
            cap[b] = (hi - lo) - pinned
            if pinned:
                tot[b] = deg0_i[ga[lo:lo + pinned]].sum()
        order = gb[np.argsort(-deg0_i[gb], kind="stable")]
        heap = [(int(tot[b]), b) for b in range(NB0) if cap[b] > 0]
        heapq.heapify(heap)
        assign = [[] for _ in range(NB0)]
        for g in order:
            t, b = heapq.heappop(heap)
            assign[b].append(g)
            cap[b] -= 1
            t += int(deg0_i[g])
            if cap[b] > 0:
                heapq.heappush(heap, (t, b))
        l2g = np.empty(NLOC, np.int64)
        l2g[:nA] = ga
        pos = nA
        for b in range(NB0):
            if assign[b]:
                l2g[pos:pos + len(assign[b])] = assign[b]
                pos += len(assign[b])
        assert pos == NLOC
        own[l2g] = c
        loc[l2g] = np.arange(NLOC)
        local2global.append(l2g)

    # ---- layer 0 buckets: (core, local block, src range) ----
    ec0 = own[dst0]
    lb0 = loc[dst0] // 128
    lp0 = loc[dst0] % 128
    rk0 = src0 // RANGE
    sl0 = (src0 - rk0 * RANGE).astype(np.int64)

    key0 = (ec0 * NB0 + lb0) * NR0 + rk0
    cnt0 = np.bincount(key0, minlength=C * NB0 * NR0).reshape(C, NB0, NR0)
    Q0 = cnt0.max(axis=0)                           # [NB0, NR0]
    seg_off0, rng_off0, rng_len0, TOT0, windows0, submm0 = _layout_ranges(
        Q0, NR0, NB0)
    NC0 = TOT0 // 128

    order0 = np.argsort(key0, kind="stable")
    s_src = sl0[order0]
    s_pos = lp0[order0]
    s_key = key0[order0]
    seg_start0 = np.searchsorted(s_key, np.arange(C * NB0 * NR0))
    seg_end0 = np.searchsorted(s_key, np.arange(C * NB0 * NR0) + 1)

    # parity of each (b, k) segment (sequence parity within range)
    par0 = np.zeros((NB0, NR0), np.int64)
    for k in range(NR0):
        seq = 0
        for b in range(NB0):
            if Q0[b, k] > 0:
                par0[b, k] = seq % 8
                seq += 1

    # ---- layer 1 buckets: (owner core of src1, global dst block) ----
    ec1 = own[src1]
    sl1 = loc[src1]
    gb1 = dst1 // 128
    gp1 = dst1 % 128
    key1 = ec1 * NB1 + gb1
    cnt1 = np.bincount(key1, minlength=C * NB1).reshape(C, NB1)
    Q1 = cnt1.max(axis=0)
    seg_off1, rng_off1, rng_len1, TOT1, windows1, submm1 = _layout_ranges(
        Q1.reshape(NB1, 1), 1, NB1)
    NC1 = TOT1 // 128

    order1 = np.argsort(key1, kind="stable")
    t_src = sl1[order1]
    t_pos = gp1[order1]
    t_key = key1[order1]
    seg_start1 = np.searchsorted(t_key, np.arange(C * NB1))
    seg_end1 = np.searchsorted(t_key, np.arange(C * NB1) + 1)
    par1 = np.zeros(NB1, np.int64)
    seq = 0
    for b in range(NB1):
        if Q1[b] > 0:
            par1[b] = seq % 8
            seq += 1

    # ---- degrees ----
    deg0 = deg0_i.astype(np.float32)
    deg1 = np.bincount(dst1, minlength=N_DST1).astype(np.float32)

    # ---- weight layouts (replicated) ----
    w0s = np.ascontiguousarray(
        np.asarray(Wself0, np.float32).reshape(2, 128, D_HID).transpose(1, 0, 2)
    ).reshape(128, 2 * D_HID)
    w0n = np.ascontiguousarray(
        np.asarray(Wneigh0, np.float32).reshape(2, 128, D_HID).transpose(1, 0, 2)
    ).reshape(128, 2 * D_HID)
    w1s = np.ascontiguousarray(
        np.asarray(Wself1, np.float32).reshape(4, 128, D_OUT).transpose(1, 0, 2)
    ).reshape(128, 4 * D_OUT)
    w1n = np.ascontiguousarray(
        np.asarray(Wneigh1, np.float32).reshape(4, 128, D_OUT).transpose(1, 0, 2)
    ).reshape(128, 4 * D_OUT)
    b0r = np.asarray(b0, np.float32).reshape(1, D_HID)
    b1r = np.asarray(b1, np.float32).reshape(1, D_OUT)
    # iota+128*m for m in 0..7, then identity
    ii = np.zeros((128, 1152), np.float32)
    for m in range(8):
        ii[:, m * 128:(m + 1) * 128] = np.arange(
            m * 128, (m + 1) * 128, dtype=np.float32)[None, :]
    ii[:, 1024:] = np.eye(128, dtype=np.float32)

    # ---- per-core data ----
    in_maps = []
    for c in range(C):
        slots_src0 = np.zeros(TOT0, np.int64)
        dlv0 = np.full(TOT0, -1.0, np.float32)
        for b in range(NB0):
            for k in range(NR0):
                if Q0[b, k] == 0:
                    continue
                kk = (c * NB0 + b) * NR0 + k
                a, e = seg_start0[kk], seg_end0[kk]
                n = e - a
                off = int(seg_off0[b, k])
                slots_src0[off:off + n] = s_src[a:e]
                dlv0[off:off + n] = (s_pos[a:e]
                                     + 128 * par0[b, k]).astype(np.float32)
        idx0 = _wrap_idx(slots_src0.astype(np.int16))
        dl0 = np.ascontiguousarray(dlv0.reshape(NC0, 128).T)   # [128, NC0]

        slots_src1 = np.zeros(TOT1, np.int64)
        dlv1 = np.full(TOT1, -1.0, np.float32)
        for b in range(NB1):
            if Q1[b] == 0:
                continue
            kk = c * NB1 + b
            a, e = seg_start1[kk], seg_end1[kk]
            n = e - a
            off = int(seg_off1[b, 0])
            slots_src1[off:off + n] = t_src[a:e]
            dlv1[off:off + n] = (t_pos[a:e] + 128 * par1[b]).astype(np.float32)
        idx1 = _wrap_idx(slots_src1.astype(np.int16))
        dl1 = np.ascontiguousarray(dlv1.reshape(NC1, 128).T)

        degloc = np.ones(NB0 * 128, np.float32)
        degloc[:NLOC] = deg0[local2global[c]]
        inv0 = np.ascontiguousarray(
            (1.0 / np.maximum(degloc, 1.0)).reshape(NB0, 128).T)

        deg1loc = np.ones(NBF * 128, np.float32)
        gl = SH1 * c + np.arange(SH1)
        valid = gl < N_DST1
        deg1loc[:SH1][valid] = deg1[gl[valid]]
        inv1 = np.ascontiguousarray(
            (1.0 / np.maximum(deg1loc, 1.0)).reshape(NBF, 128).T)

        fself = np.zeros((NB0 * 128, D_IN), np.float32)
        fself[:NLOC] = feat[local2global[c]]

        in_maps.append({
            "feat": feat, "fself": fself,
            "idx0": idx0, "dl0": dl0, "inv0": inv0,
            "idx1": idx1, "dl1": dl1, "inv1": inv1,
            "w0s": w0s, "w0n": w0n, "b0r": b0r,
            "w1s": w1s, "w1n": w1n, "b1r": b1r, "ii": ii,
        })

    # first/last nonzero range per block (for copy-vs-add into msg_sbuf)
    firstk0 = np.full(NB0, -1, np.int64)
    lastk0 = np.full(NB0, -1, np.int64)
    for b in range(NB0):
        nz = np.nonzero(Q0[b])[0]
        assert len(nz) > 0
        firstk0[b], lastk0[b] = nz[0], nz[-1]

    meta = dict(Q0=Q0, TOT0=TOT0, NC0=NC0, windows0=windows0, submm0=submm0,
                firstk0=firstk0,
                Q1=Q1, TOT1=TOT1, NC1=NC1, windows1=windows1, submm1=submm1,
                seg_off1=seg_off1,
                s_core=s_core)
    return in_maps, meta


def _build_program(meta):
    import concourse.bass as bass
    import concourse.mybir as mybir
    import concourse.tile as tile
    from concourse import bacc

    f32 = mybir.dt.float32
    i16 = mybir.dt.int16
    Q0 = meta["Q0"]
    TOT0, NC0 = meta["TOT0"], meta["NC0"]
    windows0, submm0 = meta["windows0"], meta["submm0"]
    firstk0 = meta["firstk0"]
    Q1 = meta["Q1"]
    TOT1, NC1 = meta["TOT1"], meta["NC1"]
    windows1, submm1 = meta["windows1"], meta["submm1"]

    nc = bacc.Bacc("TRN2", target_bir_lowering=False, debug=False,
                   enable_asserts=True, num_devices=C)
    t_feat = nc.dram_tensor("feat", [N_SRC0, D_IN], f32, kind="ExternalInput")
    t_fself = nc.dram_tensor("fself", [NB0 * 128, D_IN], f32, kind="ExternalInput")
    t_idx0 = nc.dram_tensor("idx0", [128, TOT0 // 16], i16, kind="ExternalInput")
    t_dl0 = nc.dram_tensor("dl0", [128, NC0], f32, kind="ExternalInput")
    t_inv0 = nc.dram_tensor("inv0", [128, NB0], f32, kind="ExternalInput")
    t_idx1 = nc.dram_tensor("idx1", [128, TOT1 // 16], i16, kind="ExternalInput")
    t_dl1 = nc.dram_tensor("dl1", [128, NC1], f32, kind="ExternalInput")
    t_inv1 = nc.dram_tensor("inv1", [128, NBF], f32, kind="ExternalInput")
    t_w0s = nc.dram_tensor("w0s", [128, 2 * D_HID], f32, kind="ExternalInput")
    t_w0n = nc.dram_tensor("w0n", [128, 2 * D_HID], f32, kind="ExternalInput")
    t_b0 = nc.dram_tensor("b0r", [1, D_HID], f32, kind="ExternalInput")
    t_w1s = nc.dram_tensor("w1s", [128, 4 * D_OUT], f32, kind="ExternalInput")
    t_w1n = nc.dram_tensor("w1n", [128, 4 * D_OUT], f32, kind="ExternalInput")
    t_b1 = nc.dram_tensor("b1r", [1, D_OUT], f32, kind="ExternalInput")
    t_ii = nc.dram_tensor("ii", [128, 1152], f32, kind="ExternalInput")
    t_out = nc.dram_tensor("out", [SH1, D_OUT], f32, kind="ExternalOutput")
    t_hmy = nc.dram_tensor("hmy", [NB0 * 128, D_HID], f32)
    t_partial = nc.dram_tensor("partial", [PAD_DST1, D_HID], f32)
    t_rs = nc.dram_tensor("rsout", [SH1, D_HID], f32)

    eq = mybir.AluOpType.is_equal
    Relu = mybir.ActivationFunctionType.Relu

    with tile.TileContext(nc) as tc:
        with tc.tile_pool(name="const", bufs=1) as cp:
            idx0_t = cp.tile([128, TOT0 // 16], i16)
            nc.sync.dma_start(idx0_t[:], t_idx0[:, :])
            dl0_t = cp.tile([128, NC0], f32)
            nc.sync.dma_start(dl0_t[:], t_dl0[:, :])
            inv0_t = cp.tile([128, NB0], f32)
            nc.sync.dma_start(inv0_t[:], t_inv0[:, :])
            idx1_t = cp.tile([128, TOT1 // 16], i16)
            nc.sync.dma_start(idx1_t[:], t_idx1[:, :])
            dl1_t = cp.tile([128, NC1], f32)
            nc.sync.dma_start(dl1_t[:], t_dl1[:, :])
            inv1_t = cp.tile([128, NBF], f32)
            nc.sync.dma_start(inv1_t[:], t_inv1[:, :])
            w0s_t = cp.tile([128, 2 * D_HID], f32)
            nc.sync.dma_start(w0s_t[:], t_w0s[:, :])
            w0n_t = cp.tile([128, 2 * D_HID], f32)
            nc.sync.dma_start(w0n_t[:], t_w0n[:, :])
            b0_t = cp.tile([1, D_HID], f32)
            nc.sync.dma_start(b0_t[:], t_b0[:, :])
            w1s_t = cp.tile([128, 4 * D_OUT], f32)
            nc.sync.dma_start(w1s_t[:], t_w1s[:, :])
            w1n_t = cp.tile([128, 4 * D_OUT], f32)
            nc.sync.dma_start(w1n_t[:], t_w1n[:, :])
            b1_t = cp.tile([1, D_OUT], f32)
            nc.sync.dma_start(b1_t[:], t_b1[:, :])
            ii_t = cp.tile([128, 1152], f32)
            nc.sync.dma_start(ii_t[:], t_ii[:, :])
            ones_t = cp.tile([1, 128], f32)
            nc.vector.memset(ones_t[:], 1.0)
            iota_par = tuple(ii_t[:, m * 128:(m + 1) * 128] for m in range(8))
            ident_t = ii_t[:, 1024:1152]
            msg_t = cp.tile([128, NB0 * D_IN], f32)   # per-block message sums

            # ================= Layer 0 =================
            with tc.tile_pool(name="g0", bufs=4) as g0p, \
                 tc.tile_pool(name="s0", bufs=12) as s0p, \
                 tc.tile_pool(name="mm0", bufs=2) as mm0p, \
                 tc.tile_pool(name="xs0", bufs=2) as xs0p, \
                 tc.tile_pool(name="xt0", bufs=2) as xt0p, \
                 tc.tile_pool(name="h0", bufs=3) as h0p, \
                 tc.tile_pool(name="pmsg", bufs=4, space="PSUM") as pmsgp, \
                 tc.tile_pool(name="pout", bufs=2, space="PSUM") as poutp, \
                 tc.tile_pool(name="pt", bufs=2, space="PSUM") as ptp:
                ctx_l0 = nc.named_scope("L0")
                ctx_l0.__enter__()
                psum_cur = {}
                for (k, sstart, slen) in windows0:
                    rbase = k * RANGE
                    rsz = min(RANGE, N_SRC0 - rbase)
                    g = g0p.tile([128, slen // 128, D_IN], f32, tag="g0")
                    nc.gpsimd.dma_gather(
                        g[:, :, :], t_feat[rbase:rbase + rsz, :],
                        idx0_t[:, sstart // 16:(sstart + slen) // 16],
                        num_idxs=slen, num_idxs_reg=slen, elem_size=D_IN)
                    for j in range(slen // 128):
                        q = sstart // 128 + j
                        for (b, par, st, sp) in submm0.get(q, []):
                            S = s0p.tile([128, 128], f32, tag="s0")
                            nc.vector.tensor_tensor(
                                out=S[:],
                                in0=dl0_t[:, q:q + 1].to_broadcast([128, 128]),
                                in1=iota_par[par], op=eq)
                            if st:
                                psum_cur[b] = pmsgp.tile(
                                    [128, D_IN], f32, tag="pmsg",
                                    name=f"pmsg_{k}_{b}")
                            nc.tensor.matmul(
                                psum_cur[b][:], lhsT=S[:], rhs=g[:, j, :],
                                start=st, stop=sp)
                            if sp:
                                mslice = msg_t[:, b * D_IN:(b + 1) * D_IN]
                                if k == firstk0[b]:
                                    nc.vector.tensor_copy(mslice, psum_cur[b][:])
                                else:
                                    nc.vector.tensor_add(mslice, mslice,
                                                         psum_cur[b][:])
                # finalize blocks: mean, self, transposes, matmuls, relu
                for b in range(NB0):
                    msgm = mm0p.tile([128, D_IN], f32)
                    nc.vector.tensor_scalar_mul(
                        msgm[:], msg_t[:, b * D_IN:(b + 1) * D_IN],
                        inv0_t[:, b:b + 1])
                    xs = xs0p.tile([128, D_IN], f32)
                    nc.sync.dma_start(xs[:], t_fself[b * 128:(b + 1) * 128, :])
                    xts = xt0p.tile([128, D_IN], f32, tag="xts")
                    xtm = xt0p.tile([128, D_IN], f32, tag="xtm")
                    for src_t, dst_t in ((xs, xts), (msgm, xtm)):
                        for kk in range(2):
                            pt = ptp.tile([128, 128], f32)
                            nc.tensor.transpose(
                                pt[:], src_t[:, kk * 128:(kk + 1) * 128],
                                ident_t)
                            nc.vector.tensor_copy(
                                dst_t[:, kk * 128:(kk + 1) * 128], pt[:])
                    pout = poutp.tile([128, D_HID], f32)
                    nc.tensor.matmul(pout[:], lhsT=xts[:, 0:128],
                                     rhs=w0s_t[:, 0:D_HID],
                                     start=True, stop=False)
                    nc.tensor.matmul(pout[:], lhsT=xts[:, 128:256],
                                     rhs=w0s_t[:, D_HID:2 * D_HID],
                                     start=False, stop=False)
                    nc.tensor.matmul(pout[:], lhsT=xtm[:, 0:128],
                                     rhs=w0n_t[:, 0:D_HID],
                                     start=False, stop=False)
                    nc.tensor.matmul(pout[:], lhsT=xtm[:, 128:256],
                                     rhs=w0n_t[:, D_HID:2 * D_HID],
                                     start=False, stop=False)
                    nc.tensor.matmul(pout[:], lhsT=ones_t[:1, :],
                                     rhs=b0_t[:1, :], start=False, stop=True)
                    h = h0p.tile([128, D_HID], f32)
                    nc.scalar.activation(h[:], pout[:], Relu)
                    nc.sync.dma_start(t_hmy[b * 128:(b + 1) * 128, :], h[:])
                ctx_l0.__exit__(None, None, None)

            # ================= Layer 1 partial sums =================
            with tc.tile_pool(name="g1", bufs=4) as g1p, \
                 tc.tile_pool(name="s1", bufs=12) as s1p, \
                 tc.tile_pool(name="pp1", bufs=3) as pp1p, \
                 tc.tile_pool(name="pm1", bufs=4, space="PSUM") as pm1p:
                ctx_l1 = nc.named_scope("L1")
                ctx_l1.__enter__()
                psum1_cur = {}
                for (_, sstart, slen) in windows1:
                    g = g1p.tile([128, slen // 128, D_HID], f32, tag="g1")
                    nc.gpsimd.dma_gather(
                        g[:, :, :], t_hmy[:, :],
                        idx1_t[:, sstart // 16:(sstart + slen) // 16],
                        num_idxs=slen, num_idxs_reg=slen, elem_size=D_HID)
                    for j in range(slen // 128):
                        q = sstart // 128 + j
                        for (b, par, st, sp) in submm1.get(q, []):
                            S = s1p.tile([128, 128], f32, tag="s1")
                            nc.vector.tensor_tensor(
                                out=S[:],
                                in0=dl1_t[:, q:q + 1].to_broadcast([128, 128]),
                                in1=iota_par[par], op=eq)
                            if st:
                                psum1_cur[b] = pm1p.tile(
                                    [128, D_HID], f32, tag="pm1",
                                    name=f"pm1_{b}")
                            nc.tensor.matmul(
                                psum1_cur[b][:], lhsT=S[:], rhs=g[:, j, :],
                                start=st, stop=sp)
                            if sp:
                                part = pp1p.tile([128, D_HID], f32)
                                nc.vector.tensor_copy(part[:], psum1_cur[b][:])
                                nc.sync.dma_start(
                                    t_partial[b * 128:(b + 1) * 128, :],
                                    part[:])
                ctx_l1.__exit__(None, None, None)

            # ================= ReduceScatter =================
            nc.gpsimd.collective_compute(
                "ReduceScatter", mybir.AluOpType.add,
                replica_groups=[list(range(C))],
                ins=[t_partial.ap().opt()], outs=[t_rs.ap().opt()])

            # ================= Final layer-1 matmuls =================
            with tc.tile_pool(name="mf", bufs=2) as mfp, \
                 tc.tile_pool(name="hsf", bufs=2) as hsfp, \
                 tc.tile_pool(name="xtf", bufs=2) as xtfp, \
                 tc.tile_pool(name="of", bufs=2) as ofp, \
                 tc.tile_pool(name="pf", bufs=2, space="PSUM") as pfp, \
                 tc.tile_pool(name="ptf", bufs=2, space="PSUM") as ptfp:
                ctx_f = nc.named_scope("FIN")
                ctx_f.__enter__()
                for fb in range(NBF):
                    nr = 128 if fb < NBF - 1 else SH1 - 128 * (NBF - 1)
                    m1 = mfp.tile([128, D_HID], f32, tag="m1")
                    nc.sync.dma_start(
                        m1[:nr, :], t_rs[fb * 128:fb * 128 + nr, :])
                    mm = mfp.tile([128, D_HID], f32, tag="mm")
                    nc.vector.tensor_scalar_mul(
                        mm[:nr, :], m1[:nr, :], inv1_t[:nr, fb:fb + 1])
                    hs = hsfp.tile([128, D_HID], f32)
                    nc.sync.dma_start(
                        hs[:nr, :], t_hmy[fb * 128:fb * 128 + nr, :])
                    xth = xtfp.tile([128, 4, 128], f32, tag="xth")
                    xtm1 = xtfp.tile([128, 4, 128], f32, tag="xtm1")
                    for src_t, dst_t in ((hs, xth), (mm, xtm1)):
                        for kk in range(4):
                            pt = ptfp.tile([128, 128], f32)
                            nc.tensor.transpose(
                                pt[:, :nr],
                                src_t[:nr, kk * 128:(kk + 1) * 128],
                                ident_t[:nr, :nr])
                            nc.vector.tensor_copy(
                                dst_t[:, kk, :nr], pt[:, :nr])
                    pf = pfp.tile([128, D_OUT], f32)
                    for kk in range(4):
                        nc.tensor.matmul(
                            pf[:nr, :], lhsT=xth[:, kk, :nr],
                            rhs=w1s_t[:, kk * D_OUT:(kk + 1) * D_OUT],
                            start=(kk == 0), stop=False)
                    for kk in range(4):
                        nc.tensor.matmul(
                            pf[:nr, :], lhsT=xtm1[:, kk, :nr],
                            rhs=w1n_t[:, kk * D_OUT:(kk + 1) * D_OUT],
                            start=False, stop=False)
                    nc.tensor.matmul(pf[:nr, :], lhsT=ones_t[:1, :nr],
                                     rhs=b1_t[:1, :], start=False, stop=True)
                    ot = ofp.tile([128, D_OUT], f32)
                    nc.vector.tensor_copy(ot[:nr, :], pf[:nr, :])
                    nc.sync.dma_start(
                        t_out[fb * 128:fb * 128 + nr, :], ot[:nr, :])
                ctx_f.__exit__(None, None, None)

    nc.compile()
    return nc


def _ensure_axon_hook():
    """Provide antenv.axon_hooks (missing in this image) so trace=True can
    capture NTFF profiles through the axon tunnel."""
    import types

    try:
        from antenv.axon_hooks import get_axon_ntff_profile_hook  # noqa: F401
        return
    except ImportError:
        pass
    mod = types.ModuleType("antenv.axon_hooks")
    _h = [None]
    mod.set_axon_ntff_profile_hook = lambda h: _h.__setitem__(0, h)
    mod.get_axon_ntff_profile_hook = lambda: _h[0]
    sys.modules["antenv.axon_hooks"] = mod
    import antenv

    antenv.axon_hooks = mod
    try:
        from trn_agent_boot.trn_boot import _ntff_profile_via_ctypes

        hook = _ntff_profile_via_ctypes("/opt/axon/libaxon_pjrt.so")
        if hook is not None:
            mod.set_axon_ntff_profile_hook(hook)
    except Exception:
        pass


def kernel(feat, Wself0, Wneigh0, b0, Wself1, Wneigh1, b1,
           edge_src0, edge_dst0, edge_src1, edge_dst1):
    global LAST_RESULT
    from concourse.bass_utils import run_bass_kernel_spmd

    if int(os.environ.get("KERNEL_TRACE", "0")):
        _ensure_axon_hook()

    in_maps, meta = _host_prep(feat, Wself0, Wneigh0, b0,
                               Wself1, Wneigh1, b1,
                               edge_src0, edge_dst0, edge_src1, edge_dst1)
    nc = _build_program(meta)
    res = run_bass_kernel_spmd(nc, in_maps, core_ids=list(range(C)),
                               trace=bool(int(os.environ.get("KERNEL_TRACE", "0"))))
    LAST_RESULT = res
    s_core = meta["s_core"]
    out = np.empty((N_DST1, D_OUT), np.float32)
    for c in range(C):
        out[SH1 * c:SH1 * c + s_core[c]] = res.results[c]["out"][:s_core[c]]
    return out
